# revision 10
# baseline (speedup 1.0000x reference)
"""Trainium2 Bass kernel: per-head (head_dim=128) Walsh-Hadamard transform.

Full input  : value [16384, 4096] f32  (= [tokens, 32 heads * 128])
Full output : same shape; out[t, h*128:(h+1)*128] = (H_128 @ v) / sqrt(128)

Strategy (pure data parallel over tokens, 8 cores, 2048 tokens each):
  - DMA in tiles of [128 tokens, 4096] (contiguous 16KB per partition).
  - Per 128x128 head block B:  Z = B @ H  needs contraction over the free
    axis, so: PE "transpose" matmul #1: B^T = matmul(lhsT=B, rhs=I,
    is_transpose=True)  -> PSUM;  DVE copies PSUM->SBUF;  PE "transpose"
    matmul #2: Z = (B^T)^T @ H = matmul(lhsT=B^T, rhs=H, is_transpose=True)
    -> PSUM (fp32 transpose-mode runs at 2 cyc/row vs 4 for plain fp32 mm).
  - ScalarE activation(Copy, scale=1/sqrt(128)) moves Z PSUM->SBUF.
  - DMA out [128, 4096] tiles.
"""

import math

import numpy as np

import concourse.bass as bass  # noqa: F401  (AP helpers)
import concourse.mybir as mybir
import concourse.tile as tile
from concourse import bacc
from concourse.bass_utils import run_bass_kernel_spmd

HEAD_DIM = 128
N_CORES = 8
TOKENS = 16384
HIDDEN = 4096
P = 128  # partitions / tile token rows


def _hadamard(n: int) -> np.ndarray:
    h = np.array([[1.0]], dtype=np.float64)
    while h.shape[0] < n:
        h = np.block([[h, h], [h, -h]])
    return h


def build_nc(tok_per_core: int = TOKENS // N_CORES, hidden: int = HIDDEN,
             group_heads: int = 4, chunk_cols: int = 2048,
             xin_bufs: int = 6, out_bufs: int = 6, xt_bufs: int = 4,
             pt_bufs: int = 4, pz_bufs: int = 4):
    """Build the per-core Bass program.

    group_heads 128-wide head blocks are batched into one PSUM bank
    ([128, group_heads*128] f32).  chunk_cols is the DMA chunk width: each
    in/out DMA moves [128, chunk_cols] f32 so the pipeline starts early and
    drains late with ~chunk-sized latency instead of full-row latency.
    """
    gw = group_heads * HEAD_DIM  # group width in columns
    assert tok_per_core % P == 0 and hidden % gw == 0
    assert chunk_cols % gw == 0 and hidden % chunk_cols == 0
    n_tiles = tok_per_core // P
    n_chunks = hidden // chunk_cols
    groups_per_chunk = chunk_cols // gw
    scale = float(np.float32(1.0 / math.sqrt(HEAD_DIM)))

    nc = bacc.Bacc("TRN2", target_bir_lowering=False)
    x = nc.dram_tensor("x", [tok_per_core, hidden], mybir.dt.float32,
                       kind="ExternalInput")
    out = nc.dram_tensor("out", [tok_per_core, hidden], mybir.dt.float32,
                         kind="ExternalOutput")
    hm = nc.inline_tensor(_hadamard(HEAD_DIM).astype(np.float32), "hm")
    ident = nc.inline_tensor(np.eye(HEAD_DIM, dtype=np.float32), "ident")

    with tile.TileContext(nc) as tc:
        with (
            tc.tile_pool(name="consts", bufs=1) as cpool,
            tc.tile_pool(name="xin", bufs=xin_bufs) as xpool,
            tc.tile_pool(name="xtb", bufs=xt_bufs) as xtpool,
            tc.tile_pool(name="outb", bufs=out_bufs) as opool,
            tc.tile_pool(name="pt", bufs=pt_bufs, space="PSUM") as ptpool,
            tc.tile_pool(name="pz", bufs=pz_bufs, space="PSUM") as pzpool,
        ):
            hm_sb = cpool.tile([HEAD_DIM, HEAD_DIM], mybir.dt.float32)
            nc.gpsimd.dma_start(hm_sb[:], hm[:])
            id_sb = cpool.tile([HEAD_DIM, HEAD_DIM], mybir.dt.float32)
            nc.gpsimd.dma_start(id_sb[:], ident[:])

            # Flat chunk schedule: graduated chunk widths — small at the very
            # start (so the first transpose begins after a tiny DMA instead
            # of 1MiB fair-shared against 5 other prefetches), ramping up to
            # chunk_cols, small again at the very end (short output drain).
            # Last-tile outputs go via the HWDGE rings so the SWDGE ring
            # drains early, off the critical path.
            sched = []  # (row, c0, width, split)
            for i in range(n_tiles):
                if i == 0:
                    w = gw
                    for ch in range(hidden // w):
                        # first two groups arrive as per-head 64KB pieces
                        sched.append((i, ch * w, w, 4 if ch < 2 else 1))
                elif i == 1:
                    w = max(gw, chunk_cols // 2)
                    for ch in range(hidden // w):
                        sched.append((i, ch * w, w, 1))
                elif i == n_tiles - 1:
                    for ch in range(hidden // gw):
                        sched.append((i, ch * gw, gw, 1))
                else:
                    for ch in range(n_chunks):
                        sched.append((i, ch * chunk_cols, chunk_cols, 1))

            for k, (i, c0, w, split) in enumerate(sched):
                x_tile = xpool.tile([P, chunk_cols], mybir.dt.float32)
                # alternate the two HWDGE rings (SP + ACT) for input
                in_eng = nc.sync if k % 2 == 0 else nc.scalar
                if split > 1:
                    # per-head mini-DMAs alternating both HWDGE rings so the
                    # first transposes start as early as possible
                    for s in range(split):
                        sw = w // split
                        eng = nc.sync if s % 2 == 0 else nc.scalar
                        eng.dma_start(
                            x_tile[:, s * sw:(s + 1) * sw],
                            x[i * P:(i + 1) * P, c0 + s * sw:c0 + (s + 1) * sw])
                else:
                    in_eng.dma_start(
                        x_tile[:, :w], x[i * P:(i + 1) * P, c0:c0 + w])
                o_tile = opool.tile([P, chunk_cols], mybir.dt.float32)
                for g in range(w // gw):
                    pt = ptpool.tile([P, gw], mybir.dt.float32)
                    for j in range(group_heads):
                        c = g * gw + j * HEAD_DIM
                        nc.tensor.transpose(
                            pt[:, j * HEAD_DIM:(j + 1) * HEAD_DIM],
                            x_tile[:, c:c + HEAD_DIM],
                            id_sb[:],
                        )
                    xt_sb = xtpool.tile([P, gw], mybir.dt.float32)
                    nc.vector.tensor_copy(xt_sb[:], pt[:])
                    pz = pzpool.tile([P, gw], mybir.dt.float32)
                    for j in range(group_heads):
                        nc.tensor.matmul(
                            pz[:, j * HEAD_DIM:(j + 1) * HEAD_DIM],
                            xt_sb[:, j * HEAD_DIM:(j + 1) * HEAD_DIM],
                            hm_sb[:],
                        )
                    nc.scalar.mul(o_tile[:, g * gw:(g + 1) * gw], pz[:],
                                  scale)
                # outputs via SWDGE (gpsimd) — separate DGE path — except
                # the final tile, which uses HWDGE for a fast tail drain
                out_eng = nc.gpsimd if i < n_tiles - 1 else (
                    nc.sync if k % 2 == 0 else nc.scalar)
                out_eng.dma_start(
                    out[i * P:(i + 1) * P, c0:c0 + w], o_tile[:, :w])
    nc.finalize()
    return nc


_NC_CACHE = {}


def _get_nc(tok_per_core: int, hidden: int):
    key = (tok_per_core, hidden)
    if key not in _NC_CACHE:
        _NC_CACHE[key] = build_nc(tok_per_core, hidden)
    return _NC_CACHE[key]


def kernel(value, **_unused) -> np.ndarray:
    value = np.ascontiguousarray(np.asarray(value), dtype=np.float32)
    tokens, hidden = value.shape
    assert tokens % N_CORES == 0
    tok_per_core = tokens // N_CORES
    nc = _get_nc(tok_per_core, hidden)
    shards = np.split(value, N_CORES, axis=0)
    in_maps = [{"x": s} for s in shards]
    res = run_bass_kernel_spmd(nc, in_maps, core_ids=list(range(N_CORES)))
    return np.concatenate([r["out"] for r in res.results], axis=0)


# revision 12
# speedup vs baseline: 1.0339x; 1.0339x over previous
"""Trainium2 Bass kernel: per-head (head_dim=128) Walsh-Hadamard transform.

Full input  : value [16384, 4096] f32  (= [tokens, 32 heads * 128])
Full output : same shape; out[t, h*128:(h+1)*128] = (H_128 @ v) / sqrt(128)

Strategy (pure data parallel over tokens, 8 cores, 2048 tokens each):
  - DMA in tiles of [128 tokens, 4096] (contiguous 16KB per partition).
  - Per 128x128 head block B:  Z = B @ H  needs contraction over the free
    axis, so: PE "transpose" matmul #1: B^T = matmul(lhsT=B, rhs=I,
    is_transpose=True)  -> PSUM;  DVE copies PSUM->SBUF;  PE "transpose"
    matmul #2: Z = (B^T)^T @ H = matmul(lhsT=B^T, rhs=H, is_transpose=True)
    -> PSUM (fp32 transpose-mode runs at 2 cyc/row vs 4 for plain fp32 mm).
  - ScalarE activation(Copy, scale=1/sqrt(128)) moves Z PSUM->SBUF.
  - DMA out [128, 4096] tiles.
"""

import math

import numpy as np

import concourse.bass as bass  # noqa: F401  (AP helpers)
import concourse.mybir as mybir
import concourse.tile as tile
from concourse import bacc
from concourse.bass_utils import run_bass_kernel_spmd

HEAD_DIM = 128
N_CORES = 8
TOKENS = 16384
HIDDEN = 4096
P = 128  # partitions / tile token rows


def _hadamard(n: int) -> np.ndarray:
    h = np.array([[1.0]], dtype=np.float64)
    while h.shape[0] < n:
        h = np.block([[h, h], [h, -h]])
    return h


def build_nc(tok_per_core: int = TOKENS // N_CORES, hidden: int = HIDDEN,
             group_heads: int = 4, chunk_cols: int = 2048,
             xin_bufs: int = 6, out_bufs: int = 6, xt_bufs: int = 4,
             pt_bufs: int = 4, pz_bufs: int = 4):
    """Build the per-core Bass program.

    group_heads 128-wide head blocks are batched into one PSUM bank
    ([128, group_heads*128] f32).  chunk_cols is the DMA chunk width: each
    in/out DMA moves [128, chunk_cols] f32 so the pipeline starts early and
    drains late with ~chunk-sized latency instead of full-row latency.
    """
    gw = group_heads * HEAD_DIM  # group width in columns
    assert tok_per_core % P == 0 and hidden % gw == 0
    assert chunk_cols % gw == 0 and hidden % chunk_cols == 0
    n_tiles = tok_per_core // P
    n_chunks = hidden // chunk_cols
    groups_per_chunk = chunk_cols // gw
    scale = float(np.float32(1.0 / math.sqrt(HEAD_DIM)))

    nc = bacc.Bacc("TRN2", target_bir_lowering=False)
    x = nc.dram_tensor("x", [tok_per_core, hidden], mybir.dt.float32,
                       kind="ExternalInput")
    out = nc.dram_tensor("out", [tok_per_core, hidden], mybir.dt.float32,
                         kind="ExternalOutput")
    hm = nc.inline_tensor(_hadamard(HEAD_DIM).astype(np.float32), "hm")
    ident = nc.inline_tensor(np.eye(HEAD_DIM, dtype=np.float32), "ident")

    with tile.TileContext(nc) as tc:
        with (
            tc.tile_pool(name="consts", bufs=1) as cpool,
            tc.tile_pool(name="xin", bufs=xin_bufs) as xpool,
            tc.tile_pool(name="xtb", bufs=xt_bufs) as xtpool,
            tc.tile_pool(name="outb", bufs=out_bufs) as opool,
            tc.tile_pool(name="pt", bufs=pt_bufs, space="PSUM") as ptpool,
            tc.tile_pool(name="pz", bufs=pz_bufs, space="PSUM") as pzpool,
        ):
            hm_sb = cpool.tile([HEAD_DIM, HEAD_DIM], mybir.dt.float32)
            nc.gpsimd.dma_start(hm_sb[:], hm[:])
            id_sb = cpool.tile([HEAD_DIM, HEAD_DIM], mybir.dt.float32)
            nc.gpsimd.dma_start(id_sb[:], ident[:])

            # Flat chunk schedule: graduated chunk widths — small at the very
            # start (so the first transpose begins after a tiny DMA instead
            # of 1MiB fair-shared against 5 other prefetches), ramping up to
            # chunk_cols, small again at the very end (short output drain).
            # Last-tile outputs go via the HWDGE rings so the SWDGE ring
            # drains early, off the critical path.
            sched = []  # (row, c0, width, split)
            for i in range(n_tiles):
                if i == 0:
                    w = gw
                    for ch in range(hidden // w):
                        # first two groups arrive as per-head 64KB pieces
                        sched.append((i, ch * w, w, 4 if ch < 2 else 1))
                elif i == 1:
                    w = max(gw, chunk_cols // 2)
                    for ch in range(hidden // w):
                        sched.append((i, ch * w, w, 1))
                else:
                    for ch in range(n_chunks):
                        sched.append((i, ch * chunk_cols, chunk_cols, 1))

            for k, (i, c0, w, split) in enumerate(sched):
                x_tile = xpool.tile([P, chunk_cols], mybir.dt.float32)
                # alternate the two HWDGE rings (SP + ACT) for input
                in_eng = nc.sync if k % 2 == 0 else nc.scalar
                if split > 1:
                    # per-head mini-DMAs alternating both HWDGE rings so the
                    # first transposes start as early as possible
                    for s in range(split):
                        sw = w // split
                        eng = nc.sync if s % 2 == 0 else nc.scalar
                        eng.dma_start(
                            x_tile[:, s * sw:(s + 1) * sw],
                            x[i * P:(i + 1) * P, c0 + s * sw:c0 + (s + 1) * sw])
                else:
                    in_eng.dma_start(
                        x_tile[:, :w], x[i * P:(i + 1) * P, c0:c0 + w])
                o_tile = opool.tile([P, chunk_cols], mybir.dt.float32)
                for g in range(w // gw):
                    pt = ptpool.tile([P, gw], mybir.dt.float32)
                    for j in range(group_heads):
                        c = g * gw + j * HEAD_DIM
                        nc.tensor.transpose(
                            pt[:, j * HEAD_DIM:(j + 1) * HEAD_DIM],
                            x_tile[:, c:c + HEAD_DIM],
                            id_sb[:],
                        )
                    xt_sb = xtpool.tile([P, gw], mybir.dt.float32)
                    nc.vector.tensor_copy(xt_sb[:], pt[:])
                    pz = pzpool.tile([P, gw], mybir.dt.float32)
                    for j in range(group_heads):
                        nc.tensor.matmul(
                            pz[:, j * HEAD_DIM:(j + 1) * HEAD_DIM],
                            xt_sb[:, j * HEAD_DIM:(j + 1) * HEAD_DIM],
                            hm_sb[:],
                        )
                    nc.scalar.mul(o_tile[:, g * gw:(g + 1) * gw], pz[:],
                                  scale)
                    if i == n_tiles - 1:
                        # final tile: drain per group via HWDGE so the last
                        # output DMA is small and the SWDGE ring is already
                        # quiet — short tail
                        eng = nc.sync if g % 2 == 0 else nc.scalar
                        eng.dma_start(
                            out[i * P:(i + 1) * P,
                                c0 + g * gw:c0 + (g + 1) * gw],
                            o_tile[:, g * gw:(g + 1) * gw])
                if i < n_tiles - 1:
                    # outputs via SWDGE (gpsimd) — separate DGE path from
                    # the two HWDGE input rings
                    nc.gpsimd.dma_start(
                        out[i * P:(i + 1) * P, c0:c0 + w], o_tile[:, :w])
    nc.finalize()
    return nc


_NC_CACHE = {}


def _get_nc(tok_per_core: int, hidden: int):
    key = (tok_per_core, hidden)
    if key not in _NC_CACHE:
        _NC_CACHE[key] = build_nc(tok_per_core, hidden)
    return _NC_CACHE[key]


def kernel(value, **_unused) -> np.ndarray:
    value = np.ascontiguousarray(np.asarray(value), dtype=np.float32)
    tokens, hidden = value.shape
    assert tokens % N_CORES == 0
    tok_per_core = tokens // N_CORES
    nc = _get_nc(tok_per_core, hidden)
    shards = np.split(value, N_CORES, axis=0)
    in_maps = [{"x": s} for s in shards]
    res = run_bass_kernel_spmd(nc, in_maps, core_ids=list(range(N_CORES)))
    return np.concatenate([r["out"] for r in res.results], axis=0)
